# revision 5
# baseline (speedup 1.0000x reference)
"""ContinuousConv (gnn_message_passing) Trainium2 kernel — 8 NeuronCores SPMD.

Contract: kernel(**inputs) takes the FULL unsharded inputs
  positions (20000,3) f32, features (20000,16) f32,
  filters (4,4,4,16,16) f32, edge_index (2,320000) i32
and returns the FULL (20000,16) f32 output of the reference scatter-mean
continuous convolution.

Strategy (self-contained; only index-level preprocessing on host):
  - Sort edges by destination row and shard CONTIGUOUS node ranges across the
    8 cores (edge counts balanced); outputs are disjoint so no all-reduce.
  - Pack each core's edges into 128-edge tiles such that no node's edge
    segment crosses a tile and each 128-node window owns exactly T tiles.
  - Device per core: dma_gather of [pos|feat] rows by col index; row
    positions reconstructed exactly via fp8 0/1 selection-matrix matmuls
    against bf16 hi/mid/lo splits of the node positions; geometry + spline
    hat weights on DVE/ACT; x1 = wa (x) (feat*window); PE transpose + K=64
    matmul against the reshaped filter table; fused multiply+reduce with the
    (b,c) hat weights; per-tile upper-triangular prefix matmul; quad-packed
    prefix dump to DRAM; per-node boundary dma_gathers + masked quad select;
    segment difference and mean by host-provided inverse degree.
"""
import os
import sys

for _p in ("/opt/trn_rl_repo", "/root/.axon_site/_ro/trn_rl_repo"):
    if os.path.isdir(_p) and _p not in sys.path:
        sys.path.insert(0, _p)

import numpy as np
import ml_dtypes
from contextlib import ExitStack

from concourse import bacc, mybir
import concourse.tile as tile
from concourse.bass_utils import run_bass_kernel_spmd
import concourse.bass as bass

F32 = mybir.dt.float32
F32R = mybir.dt.float32r
BF16 = mybir.dt.bfloat16
F8 = mybir.dt.float8e4
I16 = mybir.dt.int16
AF = mybir.ActivationFunctionType
ALU = mybir.AluOpType
P = 128
N_CORES = 8
FP8_ONE = np.float32(1.0).astype(ml_dtypes.float8_e4m3fn)
USE_F32R = os.environ.get("CCONV_F32R", "0") == "1"

# ----------------------------------------------------------------- host prep

def _pack_gather_idx(tokens, pad_val=0):
    NI = ((len(tokens) + 15) // 16) * 16
    t = np.full(NI, pad_val, np.int16)
    t[:len(tokens)] = tokens
    arr = np.zeros((128, NI // 16), np.int16)
    blk = t.reshape(NI // 16, 16).T
    for g in range(8):
        arr[g * 16:(g + 1) * 16, :] = blk
    return arr


def _prepare(edge_index, n_nodes):
    row = np.asarray(edge_index[0]).astype(np.int64)
    col = np.asarray(edge_index[1]).astype(np.int64)
    E = row.shape[0]
    order = np.argsort(row, kind="stable")
    rs = row[order]
    cs = col[order]
    seg_start = np.searchsorted(rs, np.arange(n_nodes + 1))

    target = (np.arange(1, N_CORES) * E) // N_CORES
    split_nodes = np.searchsorted(seg_start, target)
    node_lo = np.concatenate([[0], split_nodes])
    node_hi = np.concatenate([split_nodes, [n_nodes]])

    NMAX = max(node_hi[c] - node_lo[c] for c in range(N_CORES))
    NW = (NMAX + P - 1) // P
    NW = ((NW + 3) // 4) * 4
    NODE_PAD = NW * P

    packs = []
    Tmax = 1
    for c in range(N_CORES):
        lo, hi = node_lo[c], node_hi[c]
        nn = hi - lo
        degs = (seg_start[lo + 1:hi + 1] - seg_start[lo:hi]).astype(np.int64)
        assert degs.max(initial=0) <= P, "node degree exceeds 128"
        win_tiles = np.zeros(NW, np.int64)
        pos = 0
        cur_w = 0
        for i in range(nn):
            w = i // P
            if w != cur_w:
                pos = ((pos + P - 1) // P) * P
                win_tiles[cur_w] = pos // P - win_tiles[:cur_w].sum()
                cur_w = w
            if (pos % P) + degs[i] > P:
                pos = ((pos // P) + 1) * P
            pos += degs[i]
        pos_end = ((pos + P - 1) // P) * P
        win_tiles[cur_w] = pos_end // P - win_tiles[:cur_w].sum()
        Tmax = max(Tmax, int(win_tiles.max()))
        packs.append((lo, hi, degs))

    T = ((Tmax + 3) // 4) * 4
    n_tiles = NW * T
    E_pad = n_tiles * P
    NT4 = n_tiles // 4
    NJ = NW

    cores = []
    for c in range(N_CORES):
        lo, hi, degs = packs[c]
        nn = hi - lo
        starts = np.empty(nn, np.int64)
        pos = 0
        for i in range(nn):
            w = i // P
            wbase = w * T * P
            if i % P == 0:
                pos = wbase
            if (pos % P) + degs[i] > P:
                pos = ((pos // P) + 1) * P
            starts[i] = pos
            pos += degs[i]
        ends = starts + degs

        colp = np.zeros(E_pad, np.int64)
        rowp = np.full(E_pad, -1, np.int64)
        seg_ids = np.repeat(np.arange(nn), degs)
        within = np.arange(degs.sum()) - np.repeat(np.cumsum(degs) - degs, degs)
        packed_pos = starts[seg_ids] + within
        s0 = seg_start[lo]
        colp[packed_pos] = cs[s0:s0 + degs.sum()]
        rowp[packed_pos] = rs[s0:s0 + degs.sum()]

        col_idx16 = _pack_gather_idx(colp.astype(np.int16))

        selt = np.zeros((n_tiles, P, P), ml_dtypes.float8_e4m3fn)
        rl = rowp.reshape(n_tiles, P)
        tw = np.arange(n_tiles) // T
        for t in range(n_tiles):
            e = np.nonzero(rl[t] >= 0)[0]
            if len(e):
                nloc = rl[t][e] - lo - tw[t] * P
                selt[t][nloc, e] = FP8_ONE
        selt_dev = np.ascontiguousarray(selt.transpose(1, 0, 2))

        ZROW = P * NT4
        gidx_end = np.full(NODE_PAD, ZROW, np.int64)
        gidx_start = np.full(NODE_PAD, ZROW, np.int64)
        rsel_end = np.zeros(NODE_PAD, np.int64)
        rsel_start = np.zeros(NODE_PAD, np.int64)
        e1 = ends - 1
        gidx_end[:nn] = (e1 % P) * NT4 + (e1 // P) // 4
        rsel_end[:nn] = (e1 // P) % 4
        has_prev = (starts % P) != 0
        pv = starts - 1
        gidx_start[:nn] = np.where(has_prev, (pv % P) * NT4 + (pv // P) // 4, ZROW)
        rsel_start[:nn] = np.where(has_prev, (pv // P) % 4, 0)
        zd = degs == 0
        gidx_end[:nn][zd] = ZROW
        gidx_start[:nn][zd] = ZROW

        ge_idx16 = _pack_gather_idx(gidx_end.astype(np.int16))
        gs_idx16 = _pack_gather_idx(gidx_start.astype(np.int16))
        mm = np.arange(NODE_PAD)
        mE = np.zeros((P, NJ, 4), np.float32)
        mS = np.zeros((P, NJ, 4), np.float32)
        mE[mm % P, mm // P, rsel_end] = 1.0
        mS[mm % P, mm // P, rsel_start] = 1.0
        invc = np.zeros((P, NJ), np.float32)
        iv = np.zeros(NODE_PAD, np.float32)
        iv[:nn] = 1.0 / np.maximum(degs, 1).astype(np.float32)
        invc[mm % P, mm // P] = iv

        cores.append(dict(lo=lo, hi=hi, n_nodes=nn, col_idx16=col_idx16,
                          selt=selt_dev, ge_idx16=ge_idx16, gs_idx16=gs_idx16,
                          maskE=mE.reshape(P, NJ * 4), maskS=mS.reshape(P, NJ * 4),
                          invc=invc))
    meta = dict(E_pad=E_pad, n_tiles=n_tiles, NODE_PAD=NODE_PAD, NW=NW, T=T,
                NT4=NT4, G=4 * T, n_chunks=NW // 4)
    return cores, meta


def _split_pos_bf16(positions, lo_node, node_pad):
    NJ = node_pad // P
    out = np.zeros((3, P, NJ * 3), ml_dtypes.bfloat16)
    n = positions.shape[0]
    mm = np.arange(node_pad)
    src = lo_node + mm
    vals = np.zeros((node_pad, 3), np.float32)
    ok = src < n
    vals[ok] = positions[src[ok]]
    hi = vals.astype(ml_dtypes.bfloat16)
    r1 = vals - hi.astype(np.float32)
    mid = r1.astype(ml_dtypes.bfloat16)
    lo = (r1 - mid.astype(np.float32)).astype(ml_dtypes.bfloat16)
    for k, v in enumerate((hi, mid, lo)):
        out[k][(mm % P)[:, None], (mm // P)[:, None] * 3 + np.arange(3)[None, :]] = v
    return out[0], out[1], out[2]


# -------------------------------------------------------------- device build

def _build(n_nodes, n_tiles, T, node_pad, f32r=USE_F32R):
    G = 4 * T
    n_chunks = n_tiles // G
    NT4 = n_tiles // 4
    NW = node_pad // P
    NJ = NW
    NI = n_tiles * P
    nc = bacc.Bacc("TRN2", target_bir_lowering=False, debug=False, num_devices=8)

    def inp(name, shape, dt):
        return nc.dram_tensor(name, shape, dt, kind="ExternalInput").ap()

    nodedata = inp("nodedata", [n_nodes, 64], F32)
    M1 = inp("m1", [64, 256], F32)
    ramp4 = inp("ramp4", [P, 4], F32)
    colidx = inp("colidx", [P, NI // 16], I16)
    selt = inp("selt", [P, n_tiles, P], F8)
    posw = [inp(f"posw{k}", [P, NJ * 3], BF16) for k in range(3)]
    ge_idx = inp("ge_idx", [P, node_pad // 16], I16)
    gs_idx = inp("gs_idx", [P, node_pad // 16], I16)
    maskE = inp("maskE", [P, NJ * 4], F32)
    maskS = inp("maskS", [P, NJ * 4], F32)
    invc = inp("invc", [P, NJ], F32)
    ident_d = inp("identity", [P, P], F32)
    utri_d = inp("utri", [P, P], F32)
    out_ext = nc.dram_tensor("out", [P, NJ * 16], F32, kind="ExternalOutput").ap()
    pbuf = nc.dram_tensor("pbuf", [P * NT4 + 1, 64], F32).ap()

    with tile.TileContext(nc) as tc:
        with ExitStack() as ctx:
            consts = ctx.enter_context(tc.tile_pool(name="consts", bufs=1))
            pool = ctx.enter_context(tc.tile_pool(name="work", bufs=2))
            selp = ctx.enter_context(tc.tile_pool(name="selp", bufs=2))
            xtp = ctx.enter_context(tc.tile_pool(name="xtp", bufs=3))
            psum_xt = ctx.enter_context(tc.tile_pool(name="psxt", bufs=2, space="PSUM"))
            psum_h = ctx.enter_context(tc.tile_pool(name="psh", bufs=2, space="PSUM"))
            psum_p = ctx.enter_context(tc.tile_pool(name="psp", bufs=2, space="PSUM"))

            ident = consts.tile([P, P], F32)
            nc.sync.dma_start(out=ident[:], in_=ident_d[:])
            utri = consts.tile([P, P], F32)
            nc.sync.dma_start(out=utri[:], in_=utri_d[:])
            m1_sb = consts.tile([64, 256], F32)
            nc.sync.dma_start(out=m1_sb[:], in_=M1[:])
            if f32r:
                m1r = consts.tile([64, 256], F32R)
                nc.vector.tensor_copy(out=m1r[:], in_=m1_sb[:])
            ramp_sb = consts.tile([P, 4], F32)
            nc.sync.dma_start(out=ramp_sb[:], in_=ramp4[:])
            poswsb = []
            for k in range(3):
                t = consts.tile([P, NJ * 3], BF16, name=f"posw{k}", tag=f"posw{k}")
                nc.sync.dma_start(out=t[:], in_=posw[k][:])
                poswsb.append(t)
            cid = consts.tile([P, NI // 16], I16)
            nc.sync.dma_start(out=cid[:], in_=colidx[:])
            zero64 = consts.tile([1, 64], F32)
            nc.vector.memset(zero64[:], 0.0)
            nc.sync.dma_start(out=pbuf[P * NT4:P * NT4 + 1, :], in_=zero64[:])

            pbuf_v = pbuf[0:P * NT4, :].rearrange("(k q) e -> k q e", k=P)

            for ch in range(n_chunks):
                t0 = ch * G
                nd = pool.tile([P, G * 64], F32, name="nd", tag="nd")
                GSTEP = 1024
                n_g = (G * P) // GSTEP
                tpg = GSTEP // P
                for j in range(n_g):
                    nc.gpsimd.dma_gather(
                        out_ap=nd[:, j * tpg * 64:(j + 1) * tpg * 64]
                            .rearrange("p (s e) -> p s e", e=64),
                        in_ap=nodedata[:],
                        idxs_ap=cid[:, (ch * n_g + j) * (GSTEP // 16):(ch * n_g + j + 1) * (GSTEP // 16)],
                        num_idxs=GSTEP, num_idxs_reg=GSTEP, elem_size=64)
                sel_sb = selp.tile([P, G * P], F8, name="sel_sb", tag="sel_sb")
                nc.sync.dma_start(
                    out=sel_sb[:].rearrange("p (g e) -> p g e", g=G),
                    in_=selt[:, t0:t0 + G, :])

                rp = pool.tile([P, G * 3], F32, name="rp", tag="rp")
                prp = psum_p.tile([P, T * 16], F32, name="prp", tag="pacc", space="PSUM")
                for g in range(G):
                    w = ch * 4 + g // T
                    for k in range(3):
                        nc.tensor.matmul(
                            prp[:, g * 3:(g + 1) * 3],
                            lhsT=sel_sb[:, g * P:(g + 1) * P],
                            rhs=poswsb[k][:, w * 3:(w + 1) * 3],
                            start=(k == 0), stop=(k == 2))
                nc.scalar.copy(out=rp[:], in_=prp[:, 0:G * 3])

                ndv = nd[:].rearrange("p (g c) -> p g c", g=G)
                rpv = rp[:].rearrange("p (g c) -> p g c", g=G)

                sc1 = lambda tag: pool.tile([P, G], F32, name=tag, tag=tag)
                dx, dy, dz = sc1("dx"), sc1("dy"), sc1("dz")
                nc.vector.tensor_tensor(out=dx[:], in0=ndv[:, :, 0], in1=rpv[:, :, 0], op=ALU.subtract)
                nc.vector.tensor_tensor(out=dy[:], in0=ndv[:, :, 1], in1=rpv[:, :, 1], op=ALU.subtract)
                nc.vector.tensor_tensor(out=dz[:], in0=ndv[:, :, 2], in1=rpv[:, :, 2], op=ALU.subtract)
                d2, t1 = sc1("d2"), sc1("t1")
                nc.vector.tensor_tensor(out=d2[:], in0=dx[:], in1=dx[:], op=ALU.mult)
                nc.vector.tensor_tensor(out=t1[:], in0=dy[:], in1=dy[:], op=ALU.mult)
                nc.vector.tensor_add(out=d2[:], in0=d2[:], in1=t1[:])
                nc.vector.tensor_tensor(out=t1[:], in0=dz[:], in1=dz[:], op=ALU.mult)
                nc.vector.tensor_add(out=d2[:], in0=d2[:], in1=t1[:])
                u1, w_ = sc1("u1"), sc1("w")
                nc.scalar.activation(out=u1[:], in_=d2[:], func=AF.Relu, scale=-4.0, bias=1.0)
                nc.vector.tensor_tensor(out=w_[:], in0=u1[:], in1=u1[:], op=ALU.mult)
                nc.vector.tensor_tensor(out=w_[:], in0=w_[:], in1=u1[:], op=ALU.mult)
                nrm, th = sc1("nrm"), sc1("th")
                nc.scalar.activation(out=nrm[:], in_=d2[:], func=AF.Sqrt)
                nc.scalar.activation(out=th[:], in_=nrm[:], func=AF.Tanh)
                den, inv, scl = sc1("den"), sc1("inv"), sc1("scl")
                nc.vector.tensor_scalar(out=den[:], in0=nrm[:], scalar1=1e-8, scalar2=None, op0=ALU.add)
                nc.vector.reciprocal(out=inv[:], in_=den[:])
                nc.vector.tensor_tensor(out=scl[:], in0=th[:], in1=inv[:], op=ALU.mult)

                hats = []
                for dvec, tag in ((dz, "wa"), (dy, "wb"), (dx, "wc")):
                    g1 = sc1("g" + tag)
                    nc.vector.tensor_tensor(out=g1[:], in0=dvec[:], in1=scl[:], op=ALU.mult)
                    nc.vector.tensor_scalar(out=g1[:], in0=g1[:], scalar1=1.5, scalar2=1.5,
                                            op0=ALU.mult, op1=ALU.add)
                    wt = pool.tile([P, 4 * G], F32, name=tag, tag=tag)
                    nc.vector.tensor_tensor(
                        out=wt[:].rearrange("p (a g) -> p a g", a=4),
                        in0=g1[:].unsqueeze(1).broadcast_to([P, 4, G]),
                        in1=ramp_sb[:].unsqueeze(2).broadcast_to([P, 4, G]),
                        op=ALU.subtract)
                    nc.scalar.activation(out=wt[:], in_=wt[:], func=AF.Abs)
                    nc.scalar.activation(out=wt[:], in_=wt[:], func=AF.Relu, scale=-1.0, bias=1.0)
                    hats.append(wt)
                wa, wb, wc = hats

                featw = pool.tile([P, G * 16], F32, name="featw", tag="featw")
                nc.vector.tensor_tensor(
                    out=featw[:].rearrange("p (g i) -> p g i", g=G),
                    in0=ndv[:, :, 3:19],
                    in1=w_[:].unsqueeze(2).broadcast_to([P, G, 16]),
                    op=ALU.mult)

                x1 = pool.tile([P, G * 64], F32, name="x1", tag="x1", bufs=1)
                nc.vector.tensor_tensor(
                    out=x1[:].rearrange("p (g a i) -> p g a i", g=G, a=4),
                    in0=wa[:].rearrange("p (a g) -> p a g", a=4).transpose([0, 2, 1])
                        .unsqueeze(3).broadcast_to([P, G, 4, 16]),
                    in1=featw[:].rearrange("p (g i) -> p g i", g=G)
                        .unsqueeze(2).broadcast_to([P, G, 4, 16]),
                    op=ALU.mult)

                wbc = pool.tile([P, G * 16], F32, name="wbc", tag="wbc")
                nc.vector.tensor_tensor(
                    out=wbc[:].rearrange("p (g b c) -> p g b c", g=G, b=4),
                    in0=wb[:].rearrange("p (b g) -> p b g", b=4).transpose([0, 2, 1])
                        .unsqueeze(3).broadcast_to([P, G, 4, 4]),
                    in1=wc[:].rearrange("p (c g) -> p c g", c=4).transpose([0, 2, 1])
                        .unsqueeze(2).broadcast_to([P, G, 4, 4]),
                    op=ALU.mult)

                conv2 = pool.tile([P, G * 16], F32, name="conv2", tag="conv2", bufs=1)
                psb = pool.tile([P, G * 16], F32, name="psb", tag="psb", bufs=1)
                pacc = None
                for q in range(G // 4):
                    hp = psum_h.tile([P, 4 * 256], F32, name="hp", tag="hp", space="PSUM")
                    xt = psum_xt.tile([64, 4 * P], F32, name="xt", tag="xt", space="PSUM")
                    for m in range(4):
                        g = q * 4 + m
                        nc.tensor.transpose(
                            out=xt[:, m * P:(m + 1) * P],
                            in_=x1[:, 64 * g:64 * (g + 1)],
                            identity=ident[:])
                    xts = xtp.tile([64, 4 * P], F32R if f32r else F32, name="xts", tag="xts")
                    if q % 2 == 0:
                        nc.vector.tensor_copy(out=xts[:], in_=xt[:])
                    else:
                        nc.scalar.copy(out=xts[:], in_=xt[:])
                    for m in range(4):
                        nc.tensor.matmul(
                            out=hp[:, m * 256:(m + 1) * 256],
                            lhsT=xts[:, m * P:(m + 1) * P],
                            rhs=(m1r if f32r else m1_sb)[:],
                            start=True, stop=True)
                    h2t = pool.tile([P, 4 * 256], F32, name="h2t", tag="h2t")
                    nc.vector.tensor_tensor(
                        out=h2t[:].rearrange("p (q o c) -> p q o c", q=4, o=16),
                        in0=hp[:].rearrange("p (q o c) -> p q o c", q=4, o=16),
                        in1=wbc[:].rearrange("p (g c) -> p g c", g=G)[:, 4 * q:4 * q + 4, :]
                            .unsqueeze(2).broadcast_to([P, 4, 16, 16]),
                        op=ALU.mult)
                    nc.vector.tensor_reduce(
                        out=conv2[:].rearrange("p (g o) -> p g o", g=G)[:, 4 * q:4 * q + 4, :]
                            .unsqueeze(3),
                        in_=h2t[:].rearrange("p (q o c) -> p q o c", q=4, o=16),
                        axis=mybir.AxisListType.X, op=ALU.add)
                    for m in range(4):
                        g = q * 4 + m
                        go = g % T
                        if go == 0:
                            if pacc is not None:
                                wprev = (g - 1) // T
                                nc.scalar.copy(out=psb[:, wprev * T * 16:(wprev + 1) * T * 16],
                                               in_=pacc[:])
                            pacc = psum_p.tile([P, T * 16], F32, name="pacc", tag="pacc", space="PSUM")
                        nc.tensor.matmul(
                            out=pacc[:, go * 16:(go + 1) * 16],
                            lhsT=utri[:],
                            rhs=conv2[:, g * 16:(g + 1) * 16],
                            start=True, stop=True)
                nc.scalar.copy(out=psb[:, 3 * T * 16:4 * T * 16], in_=pacc[:])
                pacc = None
                nc.sync.dma_start(
                    out=pbuf_v[:, (t0 // 4):(t0 + G) // 4, :],
                    in_=psb[:].rearrange("p (q e) -> p q e", e=64))

            # extraction
            gei = consts.tile([P, node_pad // 16], I16)
            nc.sync.dma_start(out=gei[:], in_=ge_idx[:])
            gsi = consts.tile([P, node_pad // 16], I16)
            nc.sync.dma_start(out=gsi[:], in_=gs_idx[:])
            me = consts.tile([P, NJ * 4], F32)
            nc.sync.dma_start(out=me[:], in_=maskE[:])
            ms = consts.tile([P, NJ * 4], F32)
            nc.sync.dma_start(out=ms[:], in_=maskS[:])
            ic = consts.tile([P, NJ], F32)
            nc.sync.dma_start(out=ic[:], in_=invc[:])

            res = []
            for idx_t, mask_t, nm in ((gei, me, "ge4"), (gsi, ms, "gs4")):
                g4 = pool.tile([P, NJ * 64], F32, name=nm, tag=nm, bufs=1)
                off = 0
                while off < node_pad:
                    cnt = min(1024, node_pad - off)
                    nc.gpsimd.dma_gather(
                        out_ap=g4[:, (off // P) * 64:((off + cnt) // P) * 64]
                            .rearrange("p (s e) -> p s e", e=64),
                        in_ap=pbuf[:], idxs_ap=idx_t[:, off // 16:(off + cnt) // 16],
                        num_idxs=cnt, num_idxs_reg=cnt, elem_size=64)
                    off += cnt
                g4m = pool.tile([P, NJ * 64], F32, name=nm + "m", tag=nm + "m", bufs=1)
                nc.vector.tensor_tensor(
                    out=g4m[:].rearrange("p (j o r) -> p j o r", j=NJ, o=16),
                    in0=g4[:].rearrange("p (j r o) -> p j r o", j=NJ, r=4).transpose([0, 1, 3, 2]),
                    in1=mask_t[:].rearrange("p (j r) -> p j r", j=NJ)
                        .unsqueeze(2).broadcast_to([P, NJ, 16, 4]),
                    op=ALU.mult)
                g4s = pool.tile([P, NJ * 16], F32, name=nm + "s", tag=nm + "s", bufs=1)
                nc.vector.tensor_reduce(
                    out=g4s[:].rearrange("p (j o) -> p j o", j=NJ).unsqueeze(3),
                    in_=g4m[:].rearrange("p (j o r) -> p j o r", j=NJ, o=16),
                    axis=mybir.AxisListType.X, op=ALU.add)
                res.append(g4s)
            sums = pool.tile([P, NJ * 16], F32, name="sums", tag="sums", bufs=1)
            nc.vector.tensor_tensor(out=sums[:], in0=res[0][:], in1=res[1][:], op=ALU.subtract)
            outt = pool.tile([P, NJ * 16], F32, name="outt", tag="outt", bufs=1)
            nc.vector.tensor_tensor(
                out=outt[:].rearrange("p (j o) -> p j o", j=NJ),
                in0=sums[:].rearrange("p (j o) -> p j o", j=NJ),
                in1=ic[:].unsqueeze(2).broadcast_to([P, NJ, 16]),
                op=ALU.mult)
            nc.sync.dma_start(out=out_ext[:], in_=outt[:])
    nc.compile()
    return nc


_CACHE = {}
LAST_EXEC_NS = None
LAST_TRACE = None


def _install_ntff_hook():
    """Shim antenv.axon_hooks (absent in this image) and install the
    ctypes NTFF profiling hook so run_bass_kernel_spmd(trace=True)
    returns exec_time_ns. Dev-only; no-op on failure."""
    try:
        import types
        import antenv
        if "antenv.axon_hooks" not in sys.modules:
            mod = types.ModuleType("antenv.axon_hooks")
            _h = {"hook": None}
            mod.set_axon_ntff_profile_hook = lambda h: _h.__setitem__("hook", h)
            mod.get_axon_ntff_profile_hook = lambda: _h["hook"]
            sys.modules["antenv.axon_hooks"] = mod
            antenv.axon_hooks = mod
        from antenv.axon_hooks import (get_axon_ntff_profile_hook,
                                       set_axon_ntff_profile_hook)
        if get_axon_ntff_profile_hook() is None:
            from trn_agent_boot.trn_boot import _ntff_profile_via_ctypes
            hook = _ntff_profile_via_ctypes("/opt/axon/libaxon_pjrt.so")
            if hook is not None:
                set_axon_ntff_profile_hook(hook)
    except Exception:
        pass


def kernel(positions, features, filters, edge_index):
    positions = np.ascontiguousarray(np.asarray(positions, dtype=np.float32))
    features = np.ascontiguousarray(np.asarray(features, dtype=np.float32))
    filters = np.ascontiguousarray(np.asarray(filters, dtype=np.float32))
    edge_index = np.asarray(edge_index)
    n = positions.shape[0]

    cores, meta = _prepare(edge_index, n)
    key = (n, meta["n_tiles"], meta["T"], meta["NODE_PAD"])
    if key not in _CACHE:
        _CACHE[key] = _build(n, meta["n_tiles"], meta["T"], meta["NODE_PAD"])
    nc = _CACHE[key]

    nodedata = np.zeros((n, 64), np.float32)
    nodedata[:, 0:3] = positions
    nodedata[:, 3:19] = features
    M1 = np.ascontiguousarray(
        np.transpose(filters, (0, 3, 4, 1, 2)).reshape(64, 256)).astype(np.float32)
    ramp = np.tile(np.arange(4, dtype=np.float32), (P, 1))
    NODE_PAD = meta["NODE_PAD"]
    NJ = NODE_PAD // P

    in_maps = []
    for cc in cores:
        p0, p1, p2 = _split_pos_bf16(positions, cc["lo"], NODE_PAD)
        in_maps.append({
            "nodedata": nodedata, "m1": M1, "ramp4": ramp,
            "colidx": cc["col_idx16"], "selt": cc["selt"],
            "posw0": p0, "posw1": p1, "posw2": p2,
            "ge_idx": cc["ge_idx16"], "gs_idx": cc["gs_idx16"],
            "maskE": cc["maskE"], "maskS": cc["maskS"],
            "invc": cc["invc"],
            "identity": np.eye(P, dtype=np.float32),
            "utri": np.triu(np.ones((P, P), np.float32)),
        })

    do_trace = os.environ.get("CCONV_TRACE", "0") == "1"
    if do_trace:
        _install_ntff_hook()
    res = run_bass_kernel_spmd(nc, in_maps, list(range(N_CORES)), trace=do_trace)
    global LAST_EXEC_NS, LAST_TRACE
    if getattr(res, "exec_time_ns", None):
        LAST_EXEC_NS = res.exec_time_ns
        LAST_TRACE = getattr(res, "instructions_and_trace", None)

    out = np.zeros((n, 16), np.float32)
    for i, cc in enumerate(cores):
        o = res.results[i]["out"].reshape(P, NJ, 16).transpose(1, 0, 2).reshape(-1, 16)
        out[cc["lo"]:cc["hi"]] = o[:cc["n_nodes"]]
    return out



# revision 18
# speedup vs baseline: 1.5507x; 1.5507x over previous
"""ContinuousConv (gnn_message_passing) Trainium2 kernel — 8 NeuronCores SPMD.

Contract: kernel(**inputs) takes the FULL unsharded inputs
  positions (20000,3) f32, features (20000,16) f32,
  filters (4,4,4,16,16) f32, edge_index (2,320000) i32
and returns the FULL (20000,16) f32 output of the reference scatter-mean
continuous convolution.

Strategy (self-contained; only index-level preprocessing on host):
  - Sort edges by destination row and shard CONTIGUOUS node ranges across the
    8 cores (edge counts balanced); outputs are disjoint so no all-reduce.
  - Pack each core's edges into 128-edge tiles such that no node's edge
    segment crosses a tile and each 128-node window owns exactly T tiles.
  - Device per core: dma_gather of [pos|feat] rows by col index; row
    positions reconstructed exactly via fp8 0/1 selection-matrix matmuls
    against bf16 hi/mid/lo splits of the node positions; geometry + spline
    hat weights on DVE/ACT; x1 = wa (x) (feat*window); PE transpose + K=64
    matmul against the reshaped filter table; fused multiply+reduce with the
    (b,c) hat weights; per-tile upper-triangular prefix matmul; quad-packed
    prefix dump to DRAM; per-node boundary dma_gathers + masked quad select;
    segment difference and mean by host-provided inverse degree.
"""
import os
import sys

for _p in ("/opt/trn_rl_repo", "/root/.axon_site/_ro/trn_rl_repo"):
    if os.path.isdir(_p) and _p not in sys.path:
        sys.path.insert(0, _p)

import numpy as np
import ml_dtypes
from contextlib import ExitStack

from concourse import bacc, mybir
import concourse.tile as tile
from concourse.bass_utils import run_bass_kernel_spmd
import concourse.bass as bass

F32 = mybir.dt.float32
F32R = mybir.dt.float32r
BF16 = mybir.dt.bfloat16
F8 = mybir.dt.float8e4
I16 = mybir.dt.int16
AF = mybir.ActivationFunctionType
ALU = mybir.AluOpType
P = 128
N_CORES = 8
FP8_ONE = np.float32(1.0).astype(ml_dtypes.float8_e4m3fn)
USE_F32R = os.environ.get("CCONV_F32R", "0") == "1"

# ----------------------------------------------------------------- host prep

def _pack_gather_idx(tokens, pad_val=0):
    NI = ((len(tokens) + 15) // 16) * 16
    t = np.full(NI, pad_val, np.int16)
    t[:len(tokens)] = tokens
    arr = np.zeros((128, NI // 16), np.int16)
    blk = t.reshape(NI // 16, 16).T
    for g in range(8):
        arr[g * 16:(g + 1) * 16, :] = blk
    return arr


def _prepare(edge_index, n_nodes):
    row = np.asarray(edge_index[0]).astype(np.int64)
    col = np.asarray(edge_index[1]).astype(np.int64)
    E = row.shape[0]
    order = np.argsort(row, kind="stable")
    rs = row[order]
    cs = col[order]
    seg_start = np.searchsorted(rs, np.arange(n_nodes + 1))

    target = (np.arange(1, N_CORES) * E) // N_CORES
    split_nodes = np.searchsorted(seg_start, target)
    node_lo = np.concatenate([[0], split_nodes])
    node_hi = np.concatenate([split_nodes, [n_nodes]])

    NMAX = max(node_hi[c] - node_lo[c] for c in range(N_CORES))
    NW = (NMAX + P - 1) // P
    NW = ((NW + 3) // 4) * 4
    NODE_PAD = NW * P

    packs = []
    Tmax = 1
    for c in range(N_CORES):
        lo, hi = node_lo[c], node_hi[c]
        nn = hi - lo
        degs = (seg_start[lo + 1:hi + 1] - seg_start[lo:hi]).astype(np.int64)
        assert degs.max(initial=0) <= P, "node degree exceeds 128"
        win_tiles = np.zeros(NW, np.int64)
        pos = 0
        cur_w = 0
        for i in range(nn):
            w = i // P
            if w != cur_w:
                pos = ((pos + P - 1) // P) * P
                win_tiles[cur_w] = pos // P - win_tiles[:cur_w].sum()
                cur_w = w
            if (pos % P) + degs[i] > P:
                pos = ((pos // P) + 1) * P
            pos += degs[i]
        pos_end = ((pos + P - 1) // P) * P
        win_tiles[cur_w] = pos_end // P - win_tiles[:cur_w].sum()
        Tmax = max(Tmax, int(win_tiles.max()))
        packs.append((lo, hi, degs))

    T = ((Tmax + 3) // 4) * 4
    n_tiles = NW * T
    E_pad = n_tiles * P
    NT4 = n_tiles // 4
    NJ = NW

    cores = []
    for c in range(N_CORES):
        lo, hi, degs = packs[c]
        nn = hi - lo
        starts = np.empty(nn, np.int64)
        pos = 0
        for i in range(nn):
            w = i // P
            wbase = w * T * P
            if i % P == 0:
                pos = wbase
            if (pos % P) + degs[i] > P:
                pos = ((pos // P) + 1) * P
            starts[i] = pos
            pos += degs[i]
        ends = starts + degs

        colp = np.zeros(E_pad, np.int64)
        rowp = np.full(E_pad, -1, np.int64)
        seg_ids = np.repeat(np.arange(nn), degs)
        within = np.arange(degs.sum()) - np.repeat(np.cumsum(degs) - degs, degs)
        packed_pos = starts[seg_ids] + within
        s0 = seg_start[lo]
        colp[packed_pos] = cs[s0:s0 + degs.sum()]
        rowp[packed_pos] = rs[s0:s0 + degs.sum()]

        col_idx16 = _pack_gather_idx(colp.astype(np.int16))

        selt = np.zeros((n_tiles, P, P), ml_dtypes.float8_e4m3fn)
        rl = rowp.reshape(n_tiles, P)
        tw = np.arange(n_tiles) // T
        for t in range(n_tiles):
            e = np.nonzero(rl[t] >= 0)[0]
            if len(e):
                nloc = rl[t][e] - lo - tw[t] * P
                selt[t][nloc, e] = FP8_ONE
        selt_dev = np.ascontiguousarray(selt.transpose(1, 0, 2))

        ZROW = P * NT4
        gidx_end = np.full(NODE_PAD, ZROW, np.int64)
        gidx_start = np.full(NODE_PAD, ZROW, np.int64)
        rsel_end = np.zeros(NODE_PAD, np.int64)
        rsel_start = np.zeros(NODE_PAD, np.int64)
        e1 = ends - 1
        gidx_end[:nn] = (e1 % P) * NT4 + (e1 // P) // 4
        rsel_end[:nn] = (e1 // P) % 4
        has_prev = (starts % P) != 0
        pv = starts - 1
        gidx_start[:nn] = np.where(has_prev, (pv % P) * NT4 + (pv // P) // 4, ZROW)
        rsel_start[:nn] = np.where(has_prev, (pv // P) % 4, 0)
        zd = degs == 0
        gidx_end[:nn][zd] = ZROW
        gidx_start[:nn][zd] = ZROW

        ge_idx16 = _pack_gather_idx(gidx_end.astype(np.int16))
        gs_idx16 = _pack_gather_idx(gidx_start.astype(np.int16))
        mm = np.arange(NODE_PAD)
        mE = np.zeros((P, NJ, 4), np.float32)
        mS = np.zeros((P, NJ, 4), np.float32)
        mE[mm % P, mm // P, rsel_end] = 1.0
        mS[mm % P, mm // P, rsel_start] = 1.0
        invc = np.zeros((P, NJ), np.float32)
        iv = np.zeros(NODE_PAD, np.float32)
        iv[:nn] = 1.0 / np.maximum(degs, 1).astype(np.float32)
        invc[mm % P, mm // P] = iv

        cores.append(dict(lo=lo, hi=hi, n_nodes=nn, col_idx16=col_idx16,
                          selt=selt_dev, ge_idx16=ge_idx16, gs_idx16=gs_idx16,
                          maskE=mE.reshape(P, NJ * 4), maskS=mS.reshape(P, NJ * 4),
                          invc=invc))
    meta = dict(E_pad=E_pad, n_tiles=n_tiles, NODE_PAD=NODE_PAD, NW=NW, T=T,
                NT4=NT4, G=4 * T, n_chunks=NW // 4)
    return cores, meta


def _split_pos_bf16(positions, lo_node, node_pad):
    """Pack node positions as one [P, NJ*9] bf16 tensor: per window w the 9
    columns are (hi_xyz, mid_xyz, lo_xyz) so a single N=9 matmul reconstructs
    all three splits at once."""
    NJ = node_pad // P
    out = np.zeros((P, NJ * 9), ml_dtypes.bfloat16)
    n = positions.shape[0]
    mm = np.arange(node_pad)
    src = lo_node + mm
    vals = np.zeros((node_pad, 3), np.float32)
    ok = src < n
    vals[ok] = positions[src[ok]]
    hi = vals.astype(ml_dtypes.bfloat16)
    r1 = vals - hi.astype(np.float32)
    mid = r1.astype(ml_dtypes.bfloat16)
    lo = (r1 - mid.astype(np.float32)).astype(ml_dtypes.bfloat16)
    for k, v in enumerate((hi, mid, lo)):
        out[(mm % P)[:, None], (mm // P)[:, None] * 9 + k * 3 + np.arange(3)[None, :]] = v
    return out


# -------------------------------------------------------------- device build

def _build(n_nodes, n_tiles, T, node_pad, f32r=USE_F32R):
    G = 4 * T
    n_chunks = n_tiles // G
    NT4 = n_tiles // 4
    NW = node_pad // P
    NJ = NW
    NI = n_tiles * P
    nc = bacc.Bacc("TRN2", target_bir_lowering=False, debug=False, num_devices=8)

    def inp(name, shape, dt):
        return nc.dram_tensor(name, shape, dt, kind="ExternalInput").ap()

    nodedata = inp("nodedata", [n_nodes, 64], F32)
    M1 = inp("m1", [64, 256], BF16)
    ramp4 = inp("ramp4", [P, 4], F32)
    colidx = inp("colidx", [P, NI // 16], I16)
    selt = inp("selt", [P, n_tiles, P], F8)
    poswc = inp("poswc", [P, NJ * 9], BF16)
    ge_idx = inp("ge_idx", [P, node_pad // 16], I16)
    gs_idx = inp("gs_idx", [P, node_pad // 16], I16)
    maskE = inp("maskE", [P, NJ * 4], F32)
    maskS = inp("maskS", [P, NJ * 4], F32)
    invc = inp("invc", [P, NJ], F32)
    ident_d = inp("identity", [P, P], BF16)
    utri_d = inp("utri", [P, P], F8)
    out_ext = nc.dram_tensor("out", [P, NJ * 16], F32, kind="ExternalOutput").ap()
    pbuf = nc.dram_tensor("pbuf", [P * NT4 + 1, 64], F32).ap()

    with tile.TileContext(nc) as tc:
        with ExitStack() as ctx:
            consts = ctx.enter_context(tc.tile_pool(name="consts", bufs=1))
            pool = ctx.enter_context(tc.tile_pool(name="work", bufs=2))
            selp = ctx.enter_context(tc.tile_pool(name="selp", bufs=2))
            xtp = ctx.enter_context(tc.tile_pool(name="xtp", bufs=3))
            psum_xt = ctx.enter_context(tc.tile_pool(name="psxt", bufs=2, space="PSUM"))
            psum_h = ctx.enter_context(tc.tile_pool(name="psh", bufs=2, space="PSUM"))
            psum_p = ctx.enter_context(tc.tile_pool(name="psp", bufs=2, space="PSUM"))

            ident = consts.tile([P, P], BF16)
            nc.sync.dma_start(out=ident[:], in_=ident_d[:])
            utri = consts.tile([P, P], F8)
            nc.sync.dma_start(out=utri[:], in_=utri_d[:])
            m1_sb = consts.tile([64, 256], BF16)
            nc.sync.dma_start(out=m1_sb[:], in_=M1[:])
            ramp_sb = consts.tile([P, 4], F32)
            nc.sync.dma_start(out=ramp_sb[:], in_=ramp4[:])
            poswsb = consts.tile([P, NJ * 9], BF16, name="poswc", tag="poswc")
            nc.sync.dma_start(out=poswsb[:], in_=poswc[:])
            cid = consts.tile([P, NI // 16], I16)
            nc.sync.dma_start(out=cid[:], in_=colidx[:])
            zero64 = consts.tile([1, 64], F32)
            nc.vector.memset(zero64[:], 0.0)
            nc.sync.dma_start(out=pbuf[P * NT4:P * NT4 + 1, :], in_=zero64[:])

            pbuf_v = pbuf[0:P * NT4, :].rearrange("(k q) e -> k q e", k=P)

            for ch in range(n_chunks):
                t0 = ch * G
                nd = pool.tile([P, G * 64], F32, name="nd", tag="nd")
                GSTEP = 1024
                n_g = (G * P) // GSTEP
                tpg = GSTEP // P
                for j in range(n_g):
                    nc.gpsimd.dma_gather(
                        out_ap=nd[:, j * tpg * 64:(j + 1) * tpg * 64]
                            .rearrange("p (s e) -> p s e", e=64),
                        in_ap=nodedata[:],
                        idxs_ap=cid[:, (ch * n_g + j) * (GSTEP // 16):(ch * n_g + j + 1) * (GSTEP // 16)],
                        num_idxs=GSTEP, num_idxs_reg=GSTEP, elem_size=64)
                sel_sb = selp.tile([P, G * P], F8, name="sel_sb", tag="sel_sb")
                nc.sync.dma_start(
                    out=sel_sb[:].rearrange("p (g e) -> p g e", g=G),
                    in_=selt[:, t0:t0 + G, :])

                rp = pool.tile([P, G * 3], F32, name="rp", tag="rp")
                for w4 in range(4):
                    w = ch * 4 + w4
                    prp = psum_p.tile([P, T * 16], F32, name="prp", tag="pacc", space="PSUM")
                    for go in range(T):
                        g = w4 * T + go
                        nc.tensor.matmul(
                            prp[:, go * 9:(go + 1) * 9],
                            lhsT=sel_sb[:, g * P:(g + 1) * P],
                            rhs=poswsb[:, w * 9:(w + 1) * 9],
                            start=True, stop=True)
                    pv = prp[:, 0:T * 9].rearrange("p (t n) -> p t n", t=T)
                    rv = rp[:].rearrange("p (g c) -> p g c", g=G)[:, w4 * T:(w4 + 1) * T, :]
                    nc.scalar.copy(out=rv, in_=pv[:, :, 0:3])
                    nc.vector.tensor_tensor(out=rv, in0=rv, in1=pv[:, :, 3:6], op=ALU.add)
                    nc.vector.tensor_tensor(out=rv, in0=rv, in1=pv[:, :, 6:9], op=ALU.add)

                ndv = nd[:].rearrange("p (g c) -> p g c", g=G)
                rpv = rp[:].rearrange("p (g c) -> p g c", g=G)

                sc1 = lambda tag: pool.tile([P, G], F32, name=tag, tag=tag)
                dx, dy, dz = sc1("dx"), sc1("dy"), sc1("dz")
                nc.vector.tensor_tensor(out=dx[:], in0=ndv[:, :, 0], in1=rpv[:, :, 0], op=ALU.subtract)
                nc.vector.tensor_tensor(out=dy[:], in0=ndv[:, :, 1], in1=rpv[:, :, 1], op=ALU.subtract)
                nc.vector.tensor_tensor(out=dz[:], in0=ndv[:, :, 2], in1=rpv[:, :, 2], op=ALU.subtract)
                d2, t1 = sc1("d2"), sc1("t1")
                nc.vector.tensor_tensor(out=d2[:], in0=dx[:], in1=dx[:], op=ALU.mult)
                nc.vector.tensor_tensor(out=t1[:], in0=dy[:], in1=dy[:], op=ALU.mult)
                nc.vector.tensor_add(out=d2[:], in0=d2[:], in1=t1[:])
                nc.vector.tensor_tensor(out=t1[:], in0=dz[:], in1=dz[:], op=ALU.mult)
                nc.vector.tensor_add(out=d2[:], in0=d2[:], in1=t1[:])
                u1, w_ = sc1("u1"), sc1("w")
                nc.scalar.activation(out=u1[:], in_=d2[:], func=AF.Relu, scale=-4.0, bias=1.0)
                nc.vector.tensor_tensor(out=w_[:], in0=u1[:], in1=u1[:], op=ALU.mult)
                nc.vector.tensor_tensor(out=w_[:], in0=w_[:], in1=u1[:], op=ALU.mult)
                nrm, th = sc1("nrm"), sc1("th")
                nc.scalar.activation(out=nrm[:], in_=d2[:], func=AF.Sqrt)
                nc.scalar.activation(out=th[:], in_=nrm[:], func=AF.Tanh)
                den, inv, scl = sc1("den"), sc1("inv"), sc1("scl")
                nc.vector.tensor_scalar(out=den[:], in0=nrm[:], scalar1=1e-8, scalar2=None, op0=ALU.add)
                nc.vector.reciprocal(out=inv[:], in_=den[:])
                nc.vector.tensor_tensor(out=scl[:], in0=th[:], in1=inv[:], op=ALU.mult)

                hats = []
                for dvec, tag in ((dz, "wa"), (dy, "wb"), (dx, "wc")):
                    g1 = sc1("g" + tag)
                    nc.vector.tensor_tensor(out=g1[:], in0=dvec[:], in1=scl[:], op=ALU.mult)
                    nc.vector.tensor_scalar(out=g1[:], in0=g1[:], scalar1=1.5, scalar2=1.5,
                                            op0=ALU.mult, op1=ALU.add)
                    wt = pool.tile([P, 4 * G], BF16, name=tag, tag=tag)
                    nc.vector.tensor_tensor(
                        out=wt[:].rearrange("p (a g) -> p a g", a=4),
                        in0=g1[:].unsqueeze(1).broadcast_to([P, 4, G]),
                        in1=ramp_sb[:].unsqueeze(2).broadcast_to([P, 4, G]),
                        op=ALU.subtract)
                    nc.scalar.activation(out=wt[:], in_=wt[:], func=AF.Abs)
                    nc.scalar.activation(out=wt[:], in_=wt[:], func=AF.Relu, scale=-1.0, bias=1.0)
                    hats.append(wt)
                wa, wb, wc = hats

                featw = pool.tile([P, G * 16], BF16, name="featw", tag="featw")
                nc.vector.tensor_tensor(
                    out=featw[:].rearrange("p (g i) -> p g i", g=G),
                    in0=ndv[:, :, 3:19],
                    in1=w_[:].unsqueeze(2).broadcast_to([P, G, 16]),
                    op=ALU.mult)

                x1 = pool.tile([P, G * 64], BF16, name="x1", tag="x1", bufs=1)
                nc.vector.tensor_tensor(
                    out=x1[:].rearrange("p (g a i) -> p g a i", g=G, a=4),
                    in0=wa[:].rearrange("p (a g) -> p a g", a=4).transpose([0, 2, 1])
                        .unsqueeze(3).broadcast_to([P, G, 4, 16]),
                    in1=featw[:].rearrange("p (g i) -> p g i", g=G)
                        .unsqueeze(2).broadcast_to([P, G, 4, 16]),
                    op=ALU.mult)

                wbc = pool.tile([P, G * 16], BF16, name="wbc", tag="wbc")
                nc.vector.tensor_tensor(
                    out=wbc[:].rearrange("p (g b c) -> p g b c", g=G, b=4),
                    in0=wb[:].rearrange("p (b g) -> p b g", b=4).transpose([0, 2, 1])
                        .unsqueeze(3).broadcast_to([P, G, 4, 4]),
                    in1=wc[:].rearrange("p (c g) -> p c g", c=4).transpose([0, 2, 1])
                        .unsqueeze(2).broadcast_to([P, G, 4, 4]),
                    op=ALU.mult)

                conv2 = pool.tile([P, G * 16], BF16, name="conv2", tag="conv2", bufs=1)
                psb = pool.tile([P, G * 16], F32, name="psb", tag="psb", bufs=1)
                pacc = None
                for q in range(G // 4):
                    hp = psum_h.tile([P, 4 * 256], F32, name="hp", tag="hp", space="PSUM")
                    xt = psum_xt.tile([64, 4 * P], BF16, name="xt", tag="xt", space="PSUM")
                    for m in range(4):
                        g = q * 4 + m
                        nc.tensor.transpose(
                            out=xt[:, m * P:(m + 1) * P],
                            in_=x1[:, 64 * g:64 * (g + 1)],
                            identity=ident[:])
                    xts = xtp.tile([64, 4 * P], BF16, name="xts", tag="xts")
                    if q % 2 == 0:
                        nc.vector.tensor_copy(out=xts[:], in_=xt[:])
                    else:
                        nc.scalar.copy(out=xts[:], in_=xt[:])
                    for m in range(4):
                        nc.tensor.matmul(
                            out=hp[:, m * 256:(m + 1) * 256],
                            lhsT=xts[:, m * P:(m + 1) * P],
                            rhs=m1_sb[:],
                            start=True, stop=True)
                    hsb = pool.tile([P, 4 * 256], BF16, name="hsb", tag="hsb")
                    nc.scalar.copy(out=hsb[:], in_=hp[:])
                    h2t = pool.tile([P, 4 * 256], BF16, name="h2t", tag="h2t")
                    nc.vector.tensor_tensor(
                        out=h2t[:].rearrange("p (q o c) -> p q o c", q=4, o=16),
                        in0=hsb[:].rearrange("p (q o c) -> p q o c", q=4, o=16),
                        in1=wbc[:].rearrange("p (g c) -> p g c", g=G)[:, 4 * q:4 * q + 4, :]
                            .unsqueeze(2).broadcast_to([P, 4, 16, 16]),
                        op=ALU.mult)
                    with nc.allow_low_precision(reason="bc-reduce rounds once to bf16"):
                        nc.vector.tensor_reduce(
                            out=conv2[:].rearrange("p (g o) -> p g o", g=G)[:, 4 * q:4 * q + 4, :]
                                .unsqueeze(3),
                            in_=h2t[:].rearrange("p (q o c) -> p q o c", q=4, o=16),
                            axis=mybir.AxisListType.X, op=ALU.add)
                    for m in range(4):
                        g = q * 4 + m
                        go = g % T
                        if go == 0:
                            if pacc is not None:
                                wprev = (g - 1) // T
                                nc.scalar.copy(out=psb[:, wprev * T * 16:(wprev + 1) * T * 16],
                                               in_=pacc[:])
                            pacc = psum_p.tile([P, T * 16], F32, name="pacc", tag="pacc", space="PSUM")
                        nc.tensor.matmul(
                            out=pacc[:, go * 16:(go + 1) * 16],
                            lhsT=utri[:],
                            rhs=conv2[:, g * 16:(g + 1) * 16],
                            start=True, stop=True)
                nc.scalar.copy(out=psb[:, 3 * T * 16:4 * T * 16], in_=pacc[:])
                pacc = None
                nc.sync.dma_start(
                    out=pbuf_v[:, (t0 // 4):(t0 + G) // 4, :],
                    in_=psb[:].rearrange("p (q e) -> p q e", e=64))

            # extraction
            gei = consts.tile([P, node_pad // 16], I16)
            nc.sync.dma_start(out=gei[:], in_=ge_idx[:])
            gsi = consts.tile([P, node_pad // 16], I16)
            nc.sync.dma_start(out=gsi[:], in_=gs_idx[:])
            me = consts.tile([P, NJ * 4], F32)
            nc.sync.dma_start(out=me[:], in_=maskE[:])
            ms = consts.tile([P, NJ * 4], F32)
            nc.sync.dma_start(out=ms[:], in_=maskS[:])
            ic = consts.tile([P, NJ], F32)
            nc.sync.dma_start(out=ic[:], in_=invc[:])

            res = []
            for idx_t, mask_t, nm in ((gei, me, "ge4"), (gsi, ms, "gs4")):
                g4 = pool.tile([P, NJ * 64], F32, name=nm, tag=nm, bufs=1)
                off = 0
                while off < node_pad:
                    cnt = min(1024, node_pad - off)
                    nc.gpsimd.dma_gather(
                        out_ap=g4[:, (off // P) * 64:((off + cnt) // P) * 64]
                            .rearrange("p (s e) -> p s e", e=64),
                        in_ap=pbuf[:], idxs_ap=idx_t[:, off // 16:(off + cnt) // 16],
                        num_idxs=cnt, num_idxs_reg=cnt, elem_size=64)
                    off += cnt
                g4m = pool.tile([P, NJ * 64], F32, name=nm + "m", tag=nm + "m", bufs=1)
                nc.vector.tensor_tensor(
                    out=g4m[:].rearrange("p (j o r) -> p j o r", j=NJ, o=16),
                    in0=g4[:].rearrange("p (j r o) -> p j r o", j=NJ, r=4).transpose([0, 1, 3, 2]),
                    in1=mask_t[:].rearrange("p (j r) -> p j r", j=NJ)
                        .unsqueeze(2).broadcast_to([P, NJ, 16, 4]),
                    op=ALU.mult)
                g4s = pool.tile([P, NJ * 16], F32, name=nm + "s", tag=nm + "s", bufs=1)
                nc.vector.tensor_reduce(
                    out=g4s[:].rearrange("p (j o) -> p j o", j=NJ).unsqueeze(3),
                    in_=g4m[:].rearrange("p (j o r) -> p j o r", j=NJ, o=16),
                    axis=mybir.AxisListType.X, op=ALU.add)
                res.append(g4s)
            sums = pool.tile([P, NJ * 16], F32, name="sums", tag="sums", bufs=1)
            nc.vector.tensor_tensor(out=sums[:], in0=res[0][:], in1=res[1][:], op=ALU.subtract)
            outt = pool.tile([P, NJ * 16], F32, name="outt", tag="outt", bufs=1)
            nc.vector.tensor_tensor(
                out=outt[:].rearrange("p (j o) -> p j o", j=NJ),
                in0=sums[:].rearrange("p (j o) -> p j o", j=NJ),
                in1=ic[:].unsqueeze(2).broadcast_to([P, NJ, 16]),
                op=ALU.mult)
            nc.sync.dma_start(out=out_ext[:], in_=outt[:])
    nc.compile()
    return nc


_CACHE = {}
LAST_EXEC_NS = None
LAST_TRACE = None


def _install_ntff_hook():
    """Shim antenv.axon_hooks (absent in this image) and install the
    ctypes NTFF profiling hook so run_bass_kernel_spmd(trace=True)
    returns exec_time_ns. Dev-only; no-op on failure."""
    try:
        import types
        import antenv
        if "antenv.axon_hooks" not in sys.modules:
            mod = types.ModuleType("antenv.axon_hooks")
            _h = {"hook": None}
            mod.set_axon_ntff_profile_hook = lambda h: _h.__setitem__("hook", h)
            mod.get_axon_ntff_profile_hook = lambda: _h["hook"]
            sys.modules["antenv.axon_hooks"] = mod
            antenv.axon_hooks = mod
        from antenv.axon_hooks import (get_axon_ntff_profile_hook,
                                       set_axon_ntff_profile_hook)
        if get_axon_ntff_profile_hook() is None:
            from trn_agent_boot.trn_boot import _ntff_profile_via_ctypes
            hook = _ntff_profile_via_ctypes("/opt/axon/libaxon_pjrt.so")
            if hook is not None:
                set_axon_ntff_profile_hook(hook)
    except Exception:
        pass


def kernel(positions, features, filters, edge_index):
    positions = np.ascontiguousarray(np.asarray(positions, dtype=np.float32))
    features = np.ascontiguousarray(np.asarray(features, dtype=np.float32))
    filters = np.ascontiguousarray(np.asarray(filters, dtype=np.float32))
    edge_index = np.asarray(edge_index)
    n = positions.shape[0]

    cores, meta = _prepare(edge_index, n)
    key = (n, meta["n_tiles"], meta["T"], meta["NODE_PAD"])
    if key not in _CACHE:
        _CACHE[key] = _build(n, meta["n_tiles"], meta["T"], meta["NODE_PAD"])
    nc = _CACHE[key]

    nodedata = np.zeros((n, 64), np.float32)
    nodedata[:, 0:3] = positions
    nodedata[:, 3:19] = features
    M1 = np.ascontiguousarray(
        np.transpose(filters, (0, 3, 4, 1, 2)).reshape(64, 256)).astype(ml_dtypes.bfloat16)
    ramp = np.tile(np.arange(4, dtype=np.float32), (P, 1))
    NODE_PAD = meta["NODE_PAD"]
    NJ = NODE_PAD // P

    in_maps = []
    for cc in cores:
        pw = _split_pos_bf16(positions, cc["lo"], NODE_PAD)
        in_maps.append({
            "nodedata": nodedata, "m1": M1, "ramp4": ramp,
            "colidx": cc["col_idx16"], "selt": cc["selt"],
            "poswc": pw,
            "ge_idx": cc["ge_idx16"], "gs_idx": cc["gs_idx16"],
            "maskE": cc["maskE"], "maskS": cc["maskS"],
            "invc": cc["invc"],
            "identity": np.eye(P, dtype=ml_dtypes.bfloat16),
            "utri": np.triu(np.ones((P, P))).astype(ml_dtypes.float8_e4m3fn),
        })

    do_trace = os.environ.get("CCONV_TRACE", "0") == "1"
    if do_trace:
        _install_ntff_hook()
    res = run_bass_kernel_spmd(nc, in_maps, list(range(N_CORES)), trace=do_trace)
    global LAST_EXEC_NS, LAST_TRACE
    if getattr(res, "exec_time_ns", None):
        LAST_EXEC_NS = res.exec_time_ns
        LAST_TRACE = getattr(res, "instructions_and_trace", None)

    out = np.zeros((n, 16), np.float32)
    for i, cc in enumerate(cores):
        o = res.results[i]["out"].reshape(P, NJ, 16).transpose(1, 0, 2).reshape(-1, 16)
        out[cc["lo"]:cc["hi"]] = o[:cc["n_nodes"]]
    return out

